# revision 1
# baseline (speedup 1.0000x reference)
"""Trainium2 Bass kernel for fused CrossEntropy + CRL + MDCA loss.

Single-launch, data-parallel over 8 NeuronCores with on-device
cross-core reduction (collectives), so one dispatch produces the final
scalar loss.

Per core (512 rows of the 4096x32000 f32 logits):
  * stream logits in [128 x 2000] chunks (DMA ~64MB/core = the memory
    roofline), ACT computes e = exp(x) in bf16 with fused f32 row-sum
    accumulation; DVE keeps running row maxes.
  * per row-tile (4 tiles of 128 rows): DVE folds row sums s and maxes
    mx; r = 1/s; scalar engine computes lnr = ln(r) and
    conf = exp(mx + lnr) (= max-softmax confidence).
  * PE accumulates per-class column sums of p = e*r into a single PSUM
    bank across all 4 tiles (250 matmuls/tile: lhsT = e[128 rows, 128
    classes], rhs = r[128,1], start on tile 0 / stop on tile 3).
  * GPSIMD indirect-DMA gathers: logits[i, target_i], correctness[idx_i]
    and correctness[idx_{i+1 mod B}] (rolled indices precomputed on
    host; the correctness history table is replicated so the CRL roll
    needs no communication).
  * CRL conf roll crosses the core boundary only at the last local row:
    a 32B AllGather of every core's first conf value (fired mid-
    pipeline, fully hidden) supplies it; a one-hot input vector selects
    core (k+1)%8's value.
  * everything reduces into one packed [128, 252] buffer (250 cols of
    per-class sums, 1 col of CE partials, 1 col of CRL pair-term
    partials) which goes through a single AllReduce; every core then
    computes the identical final scalar (MDCA |colsum - counts|, CE and
    CRL scaling, cross-partition ones-matmul) and core 0's out_loss is
    the answer.

  Hardware sync notes (cost a day of debugging, do not regress):
    - dependent ops on the SAME engine need a semaphore self-handshake
      (inc on producer, wait before consumer): engine pipelines can read
      SBUF before the previous op's write retires.
    - input ring slots each use their OWN semaphore: with one cumulative
      DMA sem, later chunks' per-SDMA-engine increments can satisfy an
      earlier chunk's wait while that chunk is still in flight.
    - indirect DMA gathers exactly one offset per partition (the free dim
      reads consecutive elements); multi-offset-per-partition silently
      reads consecutive data on HW even though CoreSim honors it.

Host work: shard logits, flat gather offsets (targets, idx, rolled
idx), np.bincount of targets, one-hot boundary selectors. Everything
except the logits shard is packed into ONE f32 aux tensor per core
(index values < 2^24 ride exactly in f32 and gpsimd cast-DMAs them
back to int32), so a dispatch carries only 2 inputs + 1 output —
per-buffer PJRT/axon overhead was a measurable share of sustained
per-launch cost.
"""

import numpy as np

import concourse.bass as bass
from concourse import mybir
from concourse.bass_utils import run_bass_kernel_spmd

# Problem constants (hardcoded per contract).
B, C = 4096, 32000
DATASET = 50000
N_CORES = 8
R = B // N_CORES          # 512 rows per core
P = 128                   # partitions
T = R // P                # 4 row tiles per core
CW = 2000                 # column chunk width
NW = C // CW              # 16 chunks per row tile
NCH = T * NW              # 64 chunks per core
NB = 6                    # input ring buffers
CB = C // P               # 250 class blocks
PK = CB + 2               # packed allreduce width: colsum | ce | crl
K_BND = 24                # chunk index at which the boundary-conf DMA is
                          # queued (tile-0 conf is ready ~chunk 17)

# sem_v step counts in the vector end-phase (each step incs sem_v by 1;
# hardware allows only ONE sync update per DVE instruction, so consumers
# wait on these thresholds instead of dedicated semaphores)
V_BVAL = 6                # bval (boundary conf value) written
V_PACK = 14               # all three pack_sb sections written
V_U = 19                  # u vector ready for the ones-matmul
V_FIN = 20                # sc (final scalar) written

# packed aux input: hist | counts (p-major) | one-hot selector | idx3
# (int32 index values < 2^24 are exact in f32; gpsimd cast-DMAs them back)
AUX_SEL = DATASET + C
AUX_IDX = AUX_SEL + 8 * N_CORES
NAUX = AUX_IDX + P * 3 * T

FP32 = mybir.dt.float32
BF16 = mybir.dt.bfloat16
INT32 = mybir.dt.int32


def _build_merged(detect_races: bool = True,
                  debug_outs: bool = False) -> bass.Bass:
    from contextlib import ExitStack

    groups = [list(range(N_CORES))]

    nc = bass.Bass("TRN2", target_bir_lowering=False, debug=False,
                   num_devices=N_CORES,
                   detect_race_conditions=detect_races)
    xl = nc.dram_tensor("xl", [R, C], FP32, kind="ExternalInput")
    # aux packs every small input into one tensor (fewer PJRT buffers per
    # dispatch): see AUX_* offsets above
    aux = nc.dram_tensor("aux", [NAUX], FP32, kind="ExternalInput")
    out_loss = nc.dram_tensor("out_loss", [1, 1], FP32,
                              kind="ExternalOutput")
    # collective scratch (DRAM; SBUF collectives are broken on HW)
    bnd_in = nc.dram_tensor("bnd_in", [1, 8], FP32)
    bnd_out = nc.dram_tensor("bnd_out", [1, 8 * N_CORES], FP32,
                             addr_space="Shared")
    ar_in = nc.dram_tensor("ar_in", [P, PK], FP32)
    ar_out = nc.dram_tensor("ar_out", [P, PK], FP32, addr_space="Shared")
    if debug_outs:
        dbg_conf = nc.dram_tensor("dbg_conf", [P, T], FP32,
                                  kind="ExternalOutput")
        dbg_conf2 = nc.dram_tensor("dbg_conf2", [P, T], FP32,
                                   kind="ExternalOutput")
        dbg_c1 = nc.dram_tensor("dbg_c1", [P, T], FP32,
                                kind="ExternalOutput")
        dbg_c2 = nc.dram_tensor("dbg_c2", [P, T], FP32,
                                kind="ExternalOutput")
        dbg_pack = nc.dram_tensor("dbg_pack", [P, PK], FP32,
                                  kind="ExternalOutput")
        dbg_R = nc.dram_tensor("dbg_R", [P, PK], FP32,
                               kind="ExternalOutput")
        dbg_gath = nc.dram_tensor("dbg_gath", [1, 8 * N_CORES], FP32,
                                  kind="ExternalOutput")
        dbg_u = nc.dram_tensor("dbg_u", [P, 1], FP32, kind="ExternalOutput")

    xl_flat = xl.ap().rearrange("a (b c) -> (a b) c", c=1)
    hist_flat = aux.ap().rearrange("(a b) -> a b", b=1)
    cnts_src = aux[DATASET:AUX_SEL].rearrange("(p c) -> p c", c=CB)
    selv_src = aux[AUX_SEL:AUX_IDX].rearrange("(a b) -> a b",
                                              b=8 * N_CORES)
    idx3_src = aux[AUX_IDX:NAUX].rearrange("(p c) -> p c", c=3 * T)

    with ExitStack() as ctx:
        xbuf = ctx.enter_context(nc.sbuf_tensor([P, NB * CW], FP32))
        e0 = ctx.enter_context(nc.sbuf_tensor([P, C], BF16))
        e1 = ctx.enter_context(nc.sbuf_tensor([P, C], BF16))
        ebufs = [e0, e1]
        sacc = ctx.enter_context(nc.sbuf_tensor([P, NCH], FP32))
        mxp = ctx.enter_context(nc.sbuf_tensor([P, NCH], FP32))
        s_t = ctx.enter_context(nc.sbuf_tensor([P, T], FP32))
        mx_t = ctx.enter_context(nc.sbuf_tensor([P, T], FP32))
        r_t = ctx.enter_context(nc.sbuf_tensor([P, T], FP32))
        rb_t = ctx.enter_context(nc.sbuf_tensor([P, T], BF16))
        lnr_t = ctx.enter_context(nc.sbuf_tensor([P, T], FP32))
        conf_t = ctx.enter_context(nc.sbuf_tensor([P, T], FP32))
        conf2 = ctx.enter_context(nc.sbuf_tensor([P, T], FP32))
        xt_g = ctx.enter_context(nc.sbuf_tensor([P, T], FP32))
        idx3_s = ctx.enter_context(nc.sbuf_tensor([P, 3 * T], INT32))
        c1p = ctx.enter_context(nc.sbuf_tensor([P, T], FP32))
        c2p = ctx.enter_context(nc.sbuf_tensor([P, T], FP32))
        cnts_sb = ctx.enter_context(nc.sbuf_tensor([P, CB], FP32))
        selv_sb = ctx.enter_context(nc.sbuf_tensor([1, 8 * N_CORES], FP32))
        gath_sb = ctx.enter_context(nc.sbuf_tensor([1, 8 * N_CORES], FP32))
        bsel = ctx.enter_context(nc.sbuf_tensor([1, 8 * N_CORES], FP32))
        bval = ctx.enter_context(nc.sbuf_tensor([1, 1], FP32))
        zero_sb = ctx.enter_context(nc.sbuf_tensor([1, 8], FP32))
        pack_sb = ctx.enter_context(nc.sbuf_tensor([P, PK], FP32))
        R_sb = ctx.enter_context(nc.sbuf_tensor([P, PK], FP32))
        tmp_cs = ctx.enter_context(nc.sbuf_tensor([P, CB], FP32))
        s12 = ctx.enter_context(nc.sbuf_tensor([P, T], FP32))
        gtp = ctx.enter_context(nc.sbuf_tensor([P, T], FP32))
        ltp = ctx.enter_context(nc.sbuf_tensor([P, T], FP32))
        sgn = ctx.enter_context(nc.sbuf_tensor([P, T], FP32))
        dd = ctx.enter_context(nc.sbuf_tensor([P, T], FP32))
        prod = ctx.enter_context(nc.sbuf_tensor([P, T], FP32))
        terms = ctx.enter_context(nc.sbuf_tensor([P, T], FP32))
        d4 = ctx.enter_context(nc.sbuf_tensor([P, T], FP32))
        m1 = ctx.enter_context(nc.sbuf_tensor([P, 1], FP32))
        u = ctx.enter_context(nc.sbuf_tensor([P, 1], FP32))
        sc = ctx.enter_context(nc.sbuf_tensor([1, 1], FP32))
        ones_sb = ctx.enter_context(nc.sbuf_tensor([P, 1], FP32))
        psum_cs = ctx.enter_context(nc.psum_tensor([P, CB], FP32))
        psum_f = ctx.enter_context(nc.psum_tensor([1, 1], FP32))

        sems_in = [ctx.enter_context(nc.semaphore(f"sem_in{i}"))
                   for i in range(NB)]
        sem_misc = ctx.enter_context(nc.semaphore("sem_misc"))
        sem_idx = ctx.enter_context(nc.semaphore("sem_idx"))
        sem_act = ctx.enter_context(nc.semaphore("sem_act"))
        sem_dvec = ctx.enter_context(nc.semaphore("sem_dvec"))
        sem_dves = ctx.enter_context(nc.semaphore("sem_dves"))
        sem_dvs = ctx.enter_context(nc.semaphore("sem_dvs"))
        sem_scs = ctx.enter_context(nc.semaphore("sem_scs"))
        sem_acts = ctx.enter_context(nc.semaphore("sem_acts"))
        sem_pe = ctx.enter_context(nc.semaphore("sem_pe"))
        sem_gp = ctx.enter_context(nc.semaphore("sem_gp"))
        sem_z = ctx.enter_context(nc.semaphore("sem_z"))
        sem_bnd = ctx.enter_context(nc.semaphore("sem_bnd"))
        sem_ag = ctx.enter_context(nc.semaphore("sem_ag"))
        sem_gath = ctx.enter_context(nc.semaphore("sem_gath"))
        sem_sh = ctx.enter_context(nc.semaphore("sem_sh"))
        sem_ari = ctx.enter_context(nc.semaphore("sem_ari"))
        sem_ar = ctx.enter_context(nc.semaphore("sem_ar"))
        sem_R = ctx.enter_context(nc.semaphore("sem_R"))
        sem_pf = ctx.enter_context(nc.semaphore("sem_pf"))
        sem_v = ctx.enter_context(nc.semaphore("sem_v"))
        sem_od = ctx.enter_context(nc.semaphore("sem_od"))

        block = ctx.enter_context(nc.Block())

        @block.sync
        def _(sync):
            # bnd_in gather source must be fully initialized before the
            # AllGather reads it (elems 1..7 are never consumed but must
            # not be uninit DRAM).
            sync.wait_ge(sem_z, 1)
            sync.dma_start(bnd_in[:], zero_sb[:]).then_inc(sem_bnd, 16)
            for k in range(NCH):
                t, w = divmod(k, NW)
                if k == NB:
                    # off the critical start: these only feed the
                    # end-phase math, consumed much later
                    sync.dma_start(cnts_sb[:], cnts_src).then_inc(sem_misc, 16)
                    sync.dma_start(selv_sb[:], selv_src).then_inc(sem_misc, 16)
                if k == K_BND:
                    # tile-0 conf is ready around chunk 17; queue the
                    # 4-byte boundary value for the AllGather
                    sync.wait_ge(sem_acts, 1)
                    sync.dma_start(bnd_in[0:1, 0:1],
                                   conf_t[0:1, 0:1]).then_inc(sem_bnd, 16)
                if k >= NB:
                    sync.wait_ge(sem_act, k - NB + 1)
                    sync.wait_ge(sem_dvec, k - NB + 1)
                b = k % NB
                sync.dma_start(
                    xbuf[:, b * CW:(b + 1) * CW],
                    xl[t * P:(t + 1) * P, w * CW:(w + 1) * CW],
                ).then_inc(sems_in[b], 16)
            # AllGather result -> SBUF for the boundary select
            sync.wait_ge(sem_ag, 1)
            sync.dma_start(gath_sb[:], bnd_out[:]).then_inc(sem_gath, 16)
            # conf2 = roll(conf, -1) in (p, t) = row t*128+p order:
            #   conf2[p,t] = conf[p+1,t] (p<127); conf[0,t+1] (p=127,t<3);
            #   next core's conf[0,0] (p=127,t=3)
            sync.wait_ge(sem_acts, T)
            sync.dma_start(conf2[0:P - 1, 0:T],
                           conf_t[1:P, 0:T]).then_inc(sem_sh, 16)
            sync.dma_start(conf2[P - 1:P, 0:T - 1],
                           conf_t[0:1, 1:T]).then_inc(sem_sh, 16)
            sync.wait_ge(sem_v, V_BVAL)
            sync.dma_start(conf2[P - 1:P, T - 1:T],
                           bval[0:1, 0:1]).then_inc(sem_sh, 16)
            # packed partials -> AllReduce
            sync.wait_ge(sem_v, V_PACK)
            sync.dma_start(ar_in[:], pack_sb[:]).then_inc(sem_ari, 16)
            sync.wait_ge(sem_ar, 1)
            sync.dma_start(R_sb[:], ar_out[:]).then_inc(sem_R, 16)
            sync.wait_ge(sem_v, V_FIN)
            sync.dma_start(out_loss[:], sc[:]).then_inc(sem_od, 16)
            if debug_outs:
                sync.dma_start(dbg_conf[:], conf_t[:]).then_inc(sem_od, 16)
                sync.dma_start(dbg_conf2[:], conf2[:]).then_inc(sem_od, 16)
                sync.dma_start(dbg_c1[:], c1p[:]).then_inc(sem_od, 16)
                sync.dma_start(dbg_c2[:], c2p[:]).then_inc(sem_od, 16)
                sync.dma_start(dbg_pack[:], pack_sb[:]).then_inc(sem_od, 16)
                sync.dma_start(dbg_R[:], R_sb[:]).then_inc(sem_od, 16)
                sync.dma_start(dbg_gath[:], gath_sb[:]).then_inc(sem_od, 16)
                sync.dma_start(dbg_u[:], u[:]).then_inc(sem_od, 16)

        @block.scalar
        def _(scalar):
            for k in range(NCH):
                t, w = divmod(k, NW)
                if w == 0 and t >= 2:
                    # e[t%2] still being read by PE for tile t-2
                    scalar.wait_ge(sem_pe, t - 1)
                b = k % NB
                scalar.wait_ge(sems_in[b], 16 * (k // NB + 1))
                scalar.activation(
                    out=ebufs[t % 2][:, w * CW:(w + 1) * CW],
                    in_=xbuf[:, b * CW:(b + 1) * CW],
                    func=mybir.ActivationFunctionType.Exp,
                    accum_out=sacc[:, k:k + 1],
                ).then_inc(sem_act, 1)
                if w == NW - 1:
                    # tile-t stats interleaved so tile-0 conf is ready
                    # early for the boundary AllGather: lnr = ln(1/s),
                    # conf = exp(mx + lnr) = exp(mx)/s
                    scalar.wait_ge(sem_dves, t + 1)
                    scalar.activation(
                        out=lnr_t[:, t:t + 1], in_=r_t[:, t:t + 1],
                        func=mybir.ActivationFunctionType.Ln,
                    ).then_inc(sem_scs, 1)
                    scalar.wait_ge(sem_scs, t + 1)
                    scalar.activation(
                        out=conf_t[:, t:t + 1], in_=mx_t[:, t:t + 1],
                        func=mybir.ActivationFunctionType.Exp,
                        bias=lnr_t[:, t:t + 1],
                    ).then_inc(sem_acts, 1)

        @block.vector
        def _(vector):
            vector.memset(ones_sb[:], 1.0)
            vector.memset(zero_sb[:], 0.0).then_inc(sem_z, 1)
            for t in range(T):
                for w in range(NW):
                    k = t * NW + w
                    b = k % NB
                    vector.wait_ge(sems_in[b], 16 * (k // NB + 1))
                    vector.tensor_reduce(
                        out=mxp[:, k:k + 1],
                        in_=xbuf[:, b * CW:(b + 1) * CW],
                        axis=mybir.AxisListType.X,
                        op=mybir.AluOpType.max,
                    ).then_inc(sem_dvec, 1)
                # tile stats (needs ACT's sacc for this tile)
                vector.wait_ge(sem_act, NW * (t + 1))
                # self-sync: own chunk-max writes to mxp must be committed
                vector.wait_ge(sem_dvec, NW * (t + 1))
                vector.tensor_reduce(
                    out=mx_t[:, t:t + 1], in_=mxp[:, t * NW:(t + 1) * NW],
                    axis=mybir.AxisListType.X, op=mybir.AluOpType.max,
                )
                vector.tensor_reduce(
                    out=s_t[:, t:t + 1], in_=sacc[:, t * NW:(t + 1) * NW],
                    axis=mybir.AxisListType.X, op=mybir.AluOpType.add,
                ).then_inc(sem_dvs, 1)
                vector.wait_ge(sem_dvs, 2 * t + 1)
                vector.reciprocal(
                    out=r_t[:, t:t + 1], in_=s_t[:, t:t + 1]
                ).then_inc(sem_dvs, 1)
                vector.wait_ge(sem_dvs, 2 * t + 2)
                vector.tensor_copy(
                    out=rb_t[:, t:t + 1], in_=r_t[:, t:t + 1]
                ).then_inc(sem_dves, 1)

            # ---- end phase: self-handshake every dependent step ----
            n = [0]

            def step(inst):
                n[0] += 1
                inst.then_inc(sem_v, 1)
                vector.wait_ge(sem_v, n[0])

            # CRL sign terms need only the gathers
            vector.wait_ge(sem_gp, 16 * 3 * T)
            step(vector.tensor_tensor(out=s12[:], in0=c1p[:], in1=c2p[:],
                                      op=mybir.AluOpType.subtract))
            step(vector.tensor_scalar(out=gtp[:], in0=s12[:], scalar1=0.0,
                                      scalar2=None,
                                      op0=mybir.AluOpType.is_gt))
            step(vector.tensor_scalar(out=ltp[:], in0=s12[:], scalar1=0.0,
                                      scalar2=None,
                                      op0=mybir.AluOpType.is_lt))
            step(vector.tensor_tensor(out=sgn[:], in0=gtp[:], in1=ltp[:],
                                      op=mybir.AluOpType.subtract))
            # boundary conf select from the AllGather
            vector.wait_ge(sem_gath, 16)
            vector.wait_ge(sem_misc, 32)
            step(vector.tensor_tensor(out=bsel[:], in0=gath_sb[:],
                                      in1=selv_sb[:],
                                      op=mybir.AluOpType.mult))
            step(vector.tensor_reduce(
                out=bval[:], in_=bsel[:], axis=mybir.AxisListType.X,
                op=mybir.AluOpType.add,
            ))
            # CE partial: sum over rows of -(x_target - lse) = lnr + x_t
            vector.wait_ge(sem_acts, T)
            step(vector.tensor_tensor(out=d4[:], in0=lnr_t[:], in1=xt_g[:],
                                      op=mybir.AluOpType.add))
            step(vector.tensor_reduce(
                out=pack_sb[:, CB:CB + 1], in_=d4[:],
                axis=mybir.AxisListType.X, op=mybir.AluOpType.add,
            ))
            # CRL pair terms: max(0, |s12| - sgn*(conf - conf2))
            vector.wait_ge(sem_sh, 48)
            step(vector.tensor_tensor(out=dd[:], in0=conf_t[:], in1=conf2[:],
                                      op=mybir.AluOpType.subtract))
            step(vector.tensor_tensor(out=dd[:], in0=dd[:], in1=s12[:],
                                      op=mybir.AluOpType.subtract))
            step(vector.tensor_tensor(out=prod[:], in0=sgn[:], in1=dd[:],
                                      op=mybir.AluOpType.mult))
            step(vector.tensor_scalar(out=terms[:], in0=prod[:],
                                      scalar1=-1.0, scalar2=0.0,
                                      op0=mybir.AluOpType.mult,
                                      op1=mybir.AluOpType.max))
            step(vector.tensor_reduce(
                out=pack_sb[:, CB + 1:CB + 2], in_=terms[:],
                axis=mybir.AxisListType.X, op=mybir.AluOpType.add,
            ))
            # per-class sums out of PSUM
            vector.wait_ge(sem_pe, T)
            step(vector.tensor_copy(
                out=pack_sb[:, 0:CB], in_=psum_cs[:],
            ))
            # ---- post-AllReduce: identical final math on every core ----
            vector.wait_ge(sem_R, 16)
            step(vector.tensor_tensor(
                out=tmp_cs[:], in0=R_sb[:, 0:CB], in1=cnts_sb[:],
                op=mybir.AluOpType.subtract,
            ))
            step(vector.tensor_reduce(
                out=m1[:], in_=tmp_cs[:], axis=mybir.AxisListType.X,
                op=mybir.AluOpType.add, apply_absolute_value=True,
            ))
            # u = m1/(C*B) + crl/B - ce_neg/B  (col CB holds -B*loss_cls)
            step(vector.tensor_scalar_mul(u[:], m1[:], 1.0 / (C * B)))
            step(vector.scalar_tensor_tensor(
                out=u[:], in0=R_sb[:, CB + 1:CB + 2], scalar=1.0 / B,
                in1=u[:], op0=mybir.AluOpType.mult,
                op1=mybir.AluOpType.add,
            ))
            step(vector.scalar_tensor_tensor(
                out=u[:], in0=R_sb[:, CB:CB + 1], scalar=-1.0 / B,
                in1=u[:], op0=mybir.AluOpType.mult,
                op1=mybir.AluOpType.add,
            ))
            vector.wait_ge(sem_pf, 1)
            step(vector.tensor_copy(out=sc[:], in_=psum_f[0:1, 0:1]))

        @block.tensor
        def _(tensor):
            for t in range(T):
                tensor.wait_ge(sem_act, NW * (t + 1))
                tensor.wait_ge(sem_dves, t + 1)
                eb = ebufs[t % 2]
                for c in range(CB):
                    inst = tensor.matmul(
                        out=psum_cs[:, c:c + 1],
                        lhsT=eb[:, c * P:(c + 1) * P],
                        rhs=rb_t[:, t:t + 1],
                        start=(t == 0),
                        stop=(t == T - 1),
                    )
                inst.then_inc(sem_pe, 1)
            # cross-partition total of u via ones-matmul
            tensor.wait_ge(sem_v, V_U)
            tensor.matmul(
                out=psum_f[0:1, 0:1],
                lhsT=ones_sb[:, 0:1],
                rhs=u[:, 0:1],
                start=True,
                stop=True,
            ).then_inc(sem_pf, 1)

        @block.gpsimd
        def _(gpsimd):
            # cast-DMA the f32-encoded indices back to int32 (gpsimd is
            # the only engine whose DMAs may cast)
            gpsimd.dma_start(idx3_s[:], idx3_src).then_inc(sem_idx, 16)
            gpsimd.wait_ge(sem_idx, 16)
            for t in range(T):
                gpsimd.indirect_dma_start(
                    out=xt_g[:, t:t + 1],
                    out_offset=None,
                    in_=xl_flat,
                    in_offset=bass.IndirectOffsetOnAxis(
                        ap=idx3_s[:, t:t + 1], axis=0),
                ).then_inc(sem_gp, 16)
            for t in range(T):
                gpsimd.indirect_dma_start(
                    out=c1p[:, t:t + 1],
                    out_offset=None,
                    in_=hist_flat,
                    in_offset=bass.IndirectOffsetOnAxis(
                        ap=idx3_s[:, T + t:T + t + 1], axis=0),
                ).then_inc(sem_gp, 16)
            for t in range(T):
                gpsimd.indirect_dma_start(
                    out=c2p[:, t:t + 1],
                    out_offset=None,
                    in_=hist_flat,
                    in_offset=bass.IndirectOffsetOnAxis(
                        ap=idx3_s[:, 2 * T + t:2 * T + t + 1], axis=0),
                ).then_inc(sem_gp, 16)
            # 32B boundary-conf exchange, fired mid-pipeline (hidden)
            gpsimd.wait_ge(sem_bnd, 32)
            gpsimd.collective_compute(
                "AllGather",
                mybir.AluOpType.bypass,
                replica_groups=groups,
                ins=[bnd_in[:].opt()],
                outs=[bnd_out[:].opt()],
            ).then_inc(sem_ag, 1)
            # single 126KB AllReduce of all partials at the tail
            gpsimd.wait_ge(sem_ari, 16)
            gpsimd.collective_compute(
                "AllReduce",
                mybir.AluOpType.add,
                replica_groups=groups,
                ins=[ar_in[:].opt()],
                outs=[ar_out[:].opt()],
            ).then_inc(sem_ar, 1)

    return nc


_CACHE: dict[str, bass.Bass] = {}


def _get(name, builder):
    if name not in _CACHE:
        _CACHE[name] = builder()
    return _CACHE[name]


def make_in_maps(logits, targets, idx, correctness):
    """Per-core input dicts for the merged launch (host-side prep only:
    offsets, bincount, one-hot selectors)."""
    rows = np.arange(R, dtype=np.int64)
    counts = np.bincount(targets, minlength=C).astype(np.float32)
    cnts = np.ascontiguousarray(counts.reshape(CB, P).T)  # [P, CB]
    idx2 = np.roll(idx, -1)  # global roll for the CRL pairing
    aux_base = np.empty(NAUX, np.float32)
    aux_base[0:DATASET] = correctness
    aux_base[DATASET:AUX_SEL] = cnts.ravel()  # partition-major
    in_maps = []
    for k in range(N_CORES):
        sl = slice(k * R, (k + 1) * R)
        off = rows * C + targets[sl]  # flat offsets into this core's shard
        idx3 = np.empty((P, 3 * T), np.int64)
        idx3[:, 0:T] = off.reshape(T, P).T
        idx3[:, T:2 * T] = idx[sl].reshape(T, P).T
        idx3[:, 2 * T:3 * T] = idx2[sl].reshape(T, P).T
        aux = aux_base.copy()
        aux[AUX_SEL:AUX_IDX] = 0.0
        aux[AUX_SEL + 8 * ((k + 1) % N_CORES)] = 1.0
        # index values are < 2^24, exact in f32; gpsimd cast-DMAs them
        # back to int32 on device
        aux[AUX_IDX:] = idx3.ravel()
        in_maps.append({"xl": logits[sl], "aux": aux})
    return in_maps


def kernel(logits, targets, idx, correctness):
    logits = np.ascontiguousarray(np.asarray(logits, dtype=np.float32))
    targets = np.asarray(targets).astype(np.int64)
    idx = np.asarray(idx).astype(np.int64)
    correctness = np.asarray(correctness, dtype=np.float32)

    nc = _get("m", _build_merged)
    in_maps = make_in_maps(logits, targets, idx, correctness)
    res = run_bass_kernel_spmd(nc, in_maps, list(range(N_CORES)))
    total = res.results[0]["out_loss"][0, 0]
    return np.array(total, dtype=np.float32)

